# revision 36
# baseline (speedup 1.0000x reference)
"""Trainium2 Bass kernel for nn_LinearLatentKernel_84834194031187.

Computes, for x:[B,S,D], W_qkv:[3D,D], W_gate:[D,D] (fp32):
    qkv = x @ W_qkv.T + b_qkv ; q,k,v = split(qkv)
    kv_state = cumsum(k*v, axis=seq)
    out = q * kv_state * sigmoid(x @ W_gate.T + b_gate)

Sharding: 8-way channel split. Core h handles channels [h*128,(h+1)*128) of
q, k, v and the gate for ALL four batches, producing out[:, :, h*128:...].

x is host-pretransposed and pre-tiled into [NBLK, 128, B, KT, 128] fp16 so
each seq block's x^T tiles (all 4 batches) arrive via contiguous DMAs and
feed the PE stationary port directly -- no on-device transposes.

Per seq block of 128 rows (partition dim = seq):
  - one 4-bank PSUM mega-tile accumulates [k|v|q|g] x 128 channels for all
    four batches over the 8 contraction tiles (fp16 operands, fp32 PSUM,
    N=512 matmuls). Drains run at 2-batch granularity (strided PSUM APs
    across 2 banks) so each half drains on ACT/DVE while the other half's
    matmuls still run.
  - carry fold: kv[0,:] += carry as ONE [1,B,HC] DVE add; a single
    upper-triangular fp16 matmul (N=512) then yields the block cumsum
    INCLUDING the carry, and its row 127 IS the carry for the next block:
    a 1-lane scalar copy + SBUF-to-SBUF DMA moves it from partition 127
    to 0. The PE queue is software-pipelined one block.
  - out = (q * sigmoid(g)) * kv_state, one 256KB DMA per block.

Ramp: the first block's inputs are kt-sliced and spread across FOUR DMA
trigger queues (sync/scalar/gpsimd/vector) in exact MM-consumption order,
so the first real matmul starts ~2us into exec instead of ~12us. A short
burst of small dummy warmup matmuls (no DMA deps) keeps the PE HAM busy
until the data lands.

Tail: the final block drains fused, runs ONE N=512 cumsum matmul, and its
output leaves as four 64KB DMAs on four different queues.
"""

import numpy as np

import concourse.bacc as bacc
import concourse.tile as tile
import concourse.mybir as mybir
from concourse.bass_utils import run_bass_kernel_spmd

B, S, D = 4, 4096, 1024
HC = 128         # channels per core (D / 8 cores)
W4 = 4 * HC      # k|v|q|g channel block per core = 512
P = 128
NBLK = S // P    # 32 seq blocks
KT = D // P      # 8 contraction tiles

f32 = mybir.dt.float32
f16 = mybir.dt.float16

_NC_CACHE = {}


def _build(with_bias: bool):
    nc = bacc.Bacc("TRN2", target_bir_lowering=False)

    # x^T pre-tiled on host: xh[i, p, b, kt, j] = x[b, i*128+j, kt*128+p]
    xh_d = nc.dram_tensor("xh", [NBLK, P, B, KT, P], f16, kind="ExternalInput")
    # weight columns ordered [k | v | q | g], HC channels each;
    # partition-major so weight DMAs move 4KB/partition descriptors
    wt_d = nc.dram_tensor("wt", [P, KT, W4], f16, kind="ExternalInput")
    tri_d = nc.dram_tensor("tri", [P, P], f16, kind="ExternalInput")
    if with_bias:
        onesrow_d = nc.dram_tensor("onesrow", [1, P], f16, kind="ExternalInput")
        bias_d = nc.dram_tensor("bias", [1, W4], f16, kind="ExternalInput")
    # out[i, j, b, c] = result[b, i*128+j, h*128+c]; fp16 (quantization
    # adds ~2e-4 norm error vs the 2e-2 gate), host upcasts to f32
    out_d = nc.dram_tensor("out", [NBLK, P, B, HC], f16, kind="ExternalOutput")

    with tile.TileContext(nc) as tc:
        with (
            tc.tile_pool(name="consts", bufs=1) as consts,
            tc.tile_pool(name="xtp", bufs=3) as xtp,
            tc.tile_pool(name="kp", bufs=2) as kp,
            tc.tile_pool(name="gp", bufs=2) as gp,
            tc.tile_pool(name="kvp", bufs=2) as kvp,
            tc.tile_pool(name="qgp", bufs=2) as qgp,
            tc.tile_pool(name="outp", bufs=3) as outp,
            tc.tile_pool(name="tmpp", bufs=2) as tmpp,
            tc.tile_pool(name="carryp", bufs=2) as carryp,
            tc.tile_pool(name="pmm", bufs=1, space="PSUM") as pmm,
            tc.tile_pool(name="pcs_pool", bufs=2, space="PSUM") as pcs_pool,
            tc.tile_pool(name="pwm", bufs=1, space="PSUM") as pwm,
        ):
            # warmup tile memset on gpsimd: its preamble retires first, so
            # the PE can start burning warmup matmuls at the top of exec.
            # The warmup burst paces the PE until the input DMA supply can
            # sustain a continuous stream (~block 2): starting real MMs
            # earlier just stutters the stream and oscillates the HAM
            # clock gate into its half-rate state.
            warm_a = consts.tile([P, 2 * P], f16, tag="warm_a")
            nc.gpsimd.memset(warm_a[:], 0.0)
            pwarm = pwm.tile([P, 2 * P], f32, tag="pwarm")

            def warm(n):
                for _ in range(n):
                    nc.tensor.matmul(pwarm[:], warm_a[:, 0:P], warm_a[:],
                                     start=True, stop=True)

            warm(30)

            # ---- ramp: the two HWDGE queues share one ~300-400 B/ns
            # descriptor pipe, completion sems are per-DMA-piece, so land
            # (wt kt0-1 + x b0) first and follow MM consumption order:
            # b0 kt0..7, b1, b2, b3, block 1.
            xt0 = xtp.tile([P, B, KT, P], f16, tag="xt", name="xt0")
            xt1 = xtp.tile([P, B, KT, P], f16, tag="xt", name="xt1")
            wt_sb = consts.tile([P, KT, W4], f16, tag="wt")
            tri_sb = consts.tile([P, P], f16, tag="tri")

            # sync: x b0, wt 4-5, x b1, block 1 first half
            nc.sync.dma_start(xt0[:, 0:1], xh_d[0, :, 0:1])
            nc.sync.dma_start(wt_sb[:, 4:6, :], wt_d[:, 4:6, :])
            nc.sync.dma_start(xt0[:, 1:2], xh_d[0, :, 1:2])
            nc.sync.dma_start(xt1[:, 0:2], xh_d[1, :, 0:2])
            # scalar: wt 0-1, wt 2-3, wt 6-7, x b2, x b3, block 1 2nd half
            nc.scalar.dma_start(wt_sb[:, 0:2, :], wt_d[:, 0:2, :])
            nc.scalar.dma_start(wt_sb[:, 2:4, :], wt_d[:, 2:4, :])
            nc.scalar.dma_start(wt_sb[:, 6:8, :], wt_d[:, 6:8, :])
            nc.scalar.dma_start(xt0[:, 2:3], xh_d[0, :, 2:3])
            nc.scalar.dma_start(xt0[:, 3:4], xh_d[0, :, 3:4])
            nc.scalar.dma_start(xt1[:, 2:4], xh_d[1, :, 2:4])
            # gpsimd: only tri (never on the ramp or tail critical path)
            nc.gpsimd.dma_start(tri_sb[:], tri_d[:])

            if with_bias:
                onesrow_sb = consts.tile([1, P], f16, tag="onesrow")
                nc.gpsimd.dma_start(onesrow_sb[:], onesrow_d[:])
                bias_sb = consts.tile([1, W4], f16, tag="bias")
                nc.gpsimd.dma_start(bias_sb[:], bias_d[:])

            xts = {0: xt0, 1: xt1}
            pending = None      # (kvs, qgs, i) awaiting cumsum+output

            def proj_batch(ps_b, xt, b):
                for kt in range(KT):
                    nc.tensor.matmul(
                        ps_b[:], xt[:, b, kt, :], wt_sb[:, kt, :],
                        start=(kt == 0),
                        stop=(kt == KT - 1 and not with_bias),
                    )
                if with_bias:
                    nc.tensor.matmul(ps_b[:], onesrow_sb[:], bias_sb[:],
                                     start=False, stop=True)

            def cumsum_mms(pend):
                # PE part of block j's cumsum: ONE N=512 tri matmul covers all
                # four batches (columns independent); row 127 = next carry,
                # moved 127->0 via 1-lane copy + tiny SBUF DMA.
                kv_all, qg_all, j = pend
                pcs = pcs_pool.tile([P, B, HC], f32, tag="pcs")
                nc.tensor.matmul(pcs[:], tri_sb[:], kv_all[:],
                                 start=True, stop=True)
                carry_new = None
                if j < NBLK - 1:
                    tmp = tmpp.tile([P, B, HC], f16, tag="tmp")
                    nc.scalar.activation(tmp[96:P], pcs[96:P],
                                         mybir.ActivationFunctionType.Copy)
                    carry_new = carryp.tile([1, B, HC], f16, tag="carry")
                    nc.gpsimd.dma_start(carry_new[0:1], tmp[P - 1:P])
                return pcs, carry_new

            def emit_out(pend, pcs):
                _, qg_all, j = pend
                ob = outp.tile([P, B, HC], f16, tag="ob")
                nc.vector.tensor_mul(out=ob[:], in0=qg_all[:], in1=pcs[:])
                nc.sync.dma_start(out_d[j], ob[:])

            def drain_half(ps_h, kv_all, qg_all, lo, hi, last_block,
                           carry_prev):
                # ps_h: [P, 2, W4] psum pair; kv/qg products for batches
                # lo:hi as single strided ops across both banks. On the
                # final block the k copy runs on DVE (faster + no
                # cross-engine hop) while ACT does the sigmoid in parallel.
                n = hi - lo
                k_sb = kp.tile([P, n, HC], f32, tag=f"k{lo}")
                if last_block:
                    nc.vector.tensor_copy(out=k_sb[:], in_=ps_h[:, :, 0:HC])
                else:
                    nc.scalar.activation(k_sb[:], ps_h[:, :, 0:HC],
                                         mybir.ActivationFunctionType.Copy)
                nc.vector.tensor_mul(out=kv_all[:, lo:hi, :], in0=k_sb[:],
                                     in1=ps_h[:, :, HC:2 * HC])
                if last_block:
                    # final block: carry arrived long ago; folding right
                    # after the kv product unblocks the tail cumsum
                    nc.vector.tensor_add(out=kv_all[0:1, lo:hi, :],
                                         in0=kv_all[0:1, lo:hi, :],
                                         in1=carry_prev[0:1, lo:hi, :])
                g_sb = gp.tile([P, n, HC], f32, tag=f"g{lo}")
                nc.scalar.activation(g_sb[:], ps_h[:, :, 3 * HC:4 * HC],
                                     mybir.ActivationFunctionType.Sigmoid)
                nc.vector.tensor_mul(out=qg_all[:, lo:hi, :], in0=g_sb[:],
                                     in1=ps_h[:, :, 2 * HC:3 * HC])

            for i in range(NBLK):
                if i + 2 < NBLK:
                    xt = xtp.tile([P, B, KT, P], f16, tag="xt")
                    eng = nc.scalar if i % 2 == 0 else nc.sync
                    eng.dma_start(xt[:], xh_d[i + 2])
                    xts[i + 2] = xt
                xt = xts.pop(i)

                ps01 = pmm.tile([P, 2, W4], f32, tag="ps01", name="ps01")
                ps23 = pmm.tile([P, 2, W4], f32, tag="ps23", name="ps23")
                kv_all = kvp.tile([P, B, HC], f16, tag="kv")
                qg_all = qgp.tile([P, B, HC], f32, tag="qg")
                last = i == NBLK - 1

                pcs_prev = None
                if i == 0:
                    # staged order matched to the ramp DMA arrival order:
                    # b0/b1 on kt {0,1,4,5,2,3} (wt45 rides the earlier
                    # sync queue), then kt 6-7, then b2, b3. Keeps every
                    # PE wait below the ~3.4us HAM re-throttle window.
                    for b in (0, 1):
                        for kt in (0, 1, 4, 5, 2, 3):
                            nc.tensor.matmul(ps01[:, b], xt[:, b, kt, :],
                                             wt_sb[:, kt, :],
                                             start=(kt == 0), stop=False)
                    for b in (0, 1):
                        for kt in (6, 7):
                            nc.tensor.matmul(
                                ps01[:, b], xt[:, b, kt, :], wt_sb[:, kt, :],
                                start=False,
                                stop=(kt == KT - 1 and not with_bias))
                        if with_bias:
                            nc.tensor.matmul(ps01[:, b], onesrow_sb[:],
                                             bias_sb[:], start=False,
                                             stop=True)
                    proj_batch(ps23[:, 0], xt, 2)
                    proj_batch(ps23[:, 1], xt, 3)
                else:
                    proj_batch(ps01[:, 0], xt, 0)
                    proj_batch(ps01[:, 1], xt, 1)

                    # block i-1's cumsum matmuls, mid-block on the PE queue
                    if pending is not None:
                        pcs_prev, carry_prev = cumsum_mms(pending)

                    proj_batch(ps23[:, 0], xt, 2)
                    proj_batch(ps23[:, 1], xt, 3)

                if pending is not None:
                    emit_out(pending, pcs_prev)
                drain_half(ps01, kv_all, qg_all, 0, 2, last, carry_prev
                           if pending is not None else None)
                drain_half(ps23, kv_all, qg_all, 2, 4, last, carry_prev
                           if pending is not None else None)
                if 0 < i < NBLK - 1:
                    # carry fold: kv[0,:] += carry (cumsum row 127); one
                    # [1,B,HC] DVE add, after the qg drains so the PSUM
                    # drain never waits on the in-flight carry DMA
                    nc.vector.tensor_add(out=kv_all[0:1], in0=kv_all[0:1],
                                         in1=carry_prev[0:1])

                pending = (kv_all, qg_all, i)

            # final flush: two half-block chains (N=256 cumsum matmul ->
            # product -> DMA), one per HWDGE queue
            kv_all, qg_all, j = pending
            ob = outp.tile([P, B, HC], f16, tag="ob")
            # both tail DMAs on sync: scalar's stream then ends at the
            # sigmoid, so it reaches the teardown barrier ~2us earlier
            for lo, hi, eng in ((0, 2, nc.sync), (2, 4, nc.sync)):
                pcs = pcs_pool.tile([P, B, HC], f32, tag="pcs")
                nc.tensor.matmul(pcs[:, lo:hi, :], tri_sb[:],
                                 kv_all[:, lo:hi, :], start=True, stop=True)
                nc.vector.tensor_mul(out=ob[:, lo:hi, :],
                                     in0=qg_all[:, lo:hi, :],
                                     in1=pcs[:, lo:hi, :])
                eng.dma_start(out_d[j, :, lo:hi], ob[:, lo:hi, :])

    nc.compile()
    return nc


def _get_nc(with_bias: bool):
    if with_bias not in _NC_CACHE:
        _NC_CACHE[with_bias] = _build(with_bias)
    return _NC_CACHE[with_bias]


def _prep_in_maps(x, W_qkv, b_qkv, W_gate, b_gate, with_bias):
    x = np.asarray(x, dtype=np.float32).astype(np.float16)
    W_qkv = np.asarray(W_qkv, dtype=np.float32)
    W_gate = np.asarray(W_gate, dtype=np.float32)

    consts = {
        "tri": np.triu(np.ones((P, P), dtype=np.float16)),
    }
    if with_bias:
        consts["onesrow"] = np.ones((1, P), dtype=np.float16)

    # xh[i, p, b, kt, j] = x[b, i*128+j, kt*128+p]  (shared by all cores)
    xh = np.ascontiguousarray(
        x.reshape(B, NBLK, P, KT, P).transpose(1, 4, 0, 3, 2))

    in_maps = []
    for h in range(8):
        sl = slice(h * HC, (h + 1) * HC)
        wt = np.concatenate(
            [W_qkv[D + h * HC:D + (h + 1) * HC],        # k rows
             W_qkv[2 * D + h * HC:2 * D + (h + 1) * HC],  # v rows
             W_qkv[sl],                                   # q rows
             W_gate[sl]], axis=0                          # g rows
        ).T.astype(np.float16)                            # [D, 512]
        # [P, KT, W4] partition-major: 4KB/partition DMA descriptors
        wt = np.ascontiguousarray(wt.reshape(KT, P, W4).transpose(1, 0, 2))
        m = {"xh": xh, "wt": wt, **consts}
        if with_bias:
            bq = np.asarray(b_qkv, dtype=np.float32)
            bg = np.asarray(b_gate, dtype=np.float32)
            m["bias"] = np.concatenate(
                [bq[D + h * HC:D + (h + 1) * HC],
                 bq[2 * D + h * HC:2 * D + (h + 1) * HC],
                 bq[sl], bg[sl]]
            )[None, :].astype(np.float16).copy()
        in_maps.append(m)
    return in_maps


def run(x, W_qkv, b_qkv, W_gate, b_gate, trace=False, **run_kwargs):
    with_bias = bool(np.any(np.asarray(b_qkv)) or np.any(np.asarray(b_gate)))
    nc = _get_nc(with_bias)
    in_maps = _prep_in_maps(x, W_qkv, b_qkv, W_gate, b_gate, with_bias)
    res = run_bass_kernel_spmd(nc, in_maps, list(range(8)), trace=trace, **run_kwargs)
    out = np.empty((B, S, D), dtype=np.float32)
    for h in range(8):
        # res[h]["out"]: [NBLK, P, B, HC] f16 -> out[b, s, h*HC:(h+1)*HC]
        o = np.asarray(res.results[h]["out"]).astype(np.float32)
        o = o.transpose(2, 0, 1, 3)
        out[:, :, h * HC:(h + 1) * HC] = o.reshape(B, S, HC)
    return out, res


def kernel(x, W_qkv, b_qkv, W_gate, b_gate):
    out, _ = run(x, W_qkv, b_qkv, W_gate, b_gate)
    return out
